# revision 16
# baseline (speedup 1.0000x reference)
"""Distributed GQA attention kernel for one TRN2 chip (8 NeuronCores).

nn_Attention: B=2, S=2048, D=2048, H=32 q-heads, KV=8 kv-heads, HD=64,
RoPE (interleaved pairs), causal softmax, GQA repeat 4, output proj.

Sharding (tensor-parallel over heads): core c owns q-heads 4c..4c+3 and
kv-head c. x and freq tables replicated. Instead of an AllReduce after wo,
each core's per-head attention output is exchanged with an AllToAll (bf16,
1/16 the AllReduce bytes) so that core c ends up with the full attention
activation for tokens [256c:256c+256) of each batch, then computes the wo
projection for just those tokens. Host concatenates the 8 token slices.

Device pipeline per core:
  P1: x --(cast-DMA bf16)--> SBUF, PE-transpose to d-major xT; fused QKV
      matmuls (host-transposed / RoPE-deinterleaved weights); RoPE on DVE
      directly from PSUM; V transposed to token-major with a ones column
      appended (rowsum trick).
  P2: per (batch, head): scores^T = K^T q with K stationary (S.T
      orientation), exp on ACT straight from PSUM (no max subtraction --
      |scores| < 6 for this problem's scale), causal zeroing of the
      diagonal block via gpsimd affine_select, PV with expS^T stationary
      and V_aug moving (65th column accumulates the softmax denominator),
      reciprocal + scale, PE-transpose to e-major, DMA into A2A chunks.
  P3: per batch: AllToAll, then wo matmul with received activation
      stationary and host-transposed wo moving -> token-major output.
"""
from contextlib import ExitStack

import numpy as np

import concourse.bass as bass
import concourse.mybir as mybir
import concourse.tile as tile
from concourse import bacc
from concourse.bass_utils import run_bass_kernel_spmd
from concourse.masks import make_identity

F32 = mybir.dt.float32
BF16 = mybir.dt.bfloat16
AF = mybir.ActivationFunctionType

NC_CORES = 8
B = 2
S = 2048
D = 2048
H = 32
KV = 8
HD = 64
HPC = H // NC_CORES      # 4 q heads per core
EQ = HPC * HD            # 256
T = B * S
TB = 512                 # phase-1 token block
NTB = T // TB
KTILES = S // 128
DT = D // 128
TSLICE = T // NC_CORES
BSL = TSLICE // B        # per-batch token slice each core outputs
QSPAN = 1024


def build():
    nc = bacc.Bacc("TRN2", target_bir_lowering=False, debug=False,
                   num_devices=NC_CORES)

    x = nc.dram_tensor("x", [T, D], F32, kind="ExternalInput")
    cos4 = nc.dram_tensor("cos4", [128, S], F32, kind="ExternalInput")
    sin4 = nc.dram_tensor("sin4", [128, S], F32, kind="ExternalInput")
    wqTA = nc.dram_tensor("wqTA", [D, 128], F32, kind="ExternalInput")
    wqTB = nc.dram_tensor("wqTB", [D, 128], F32, kind="ExternalInput")
    wkvT = nc.dram_tensor("wkvT", [D, 128], F32, kind="ExternalInput")
    woT = nc.dram_tensor("woT", [D, D], F32, kind="ExternalInput")
    out = nc.dram_tensor("out", [TSLICE, D], F32, kind="ExternalOutput")

    a2a_in = [nc.dram_tensor(f"a2a_in{b}", [NC_CORES, EQ, BSL], BF16)
              for b in range(B)]
    a2a_out = [nc.dram_tensor(f"a2a_out{b}", [NC_CORES, EQ, BSL], BF16)
               for b in range(B)]
    rg = [list(range(NC_CORES))]

    with tile.TileContext(nc) as tc, ExitStack() as es:
        const = es.enter_context(tc.tile_pool(name="const", bufs=1))
        ident = const.tile([128, 128], BF16, tag="ident")
        make_identity(nc, ident[:])

        qt_pool = es.enter_context(tc.tile_pool(name="qt", bufs=1))
        QT = [qt_pool.tile([64, T], BF16, tag=f"QT{i}", name=f"QT{i}")
              for i in range(HPC)]
        KT = qt_pool.tile([64, T], BF16, tag="KT")

        vpool = es.enter_context(tc.tile_pool(name="vaug", bufs=B * KTILES))
        V_aug = []
        for i in range(B * KTILES):
            v = vpool.tile([128, 65], BF16, tag="vaug")
            nc.gpsimd.memset(v[:, 64:65], 1.0)
            V_aug.append(v)

        # ---------- phase 1: transpose x, QKV, rope ----------
        with tc.tile_pool(name="p1c", bufs=1) as p1c, \
             tc.tile_pool(name="p1sb", bufs=3) as p1sb, \
             tc.tile_pool(name="xbfp", bufs=8) as xbfp, \
             tc.tile_pool(name="xtp", bufs=4) as xtp, \
             tc.tile_pool(name="p1ps", bufs=2, space="PSUM") as p1ps, \
             tc.tile_pool(name="p1pst", bufs=2, space="PSUM") as p1pst:
            cos_sb = p1c.tile([128, S], F32, tag="cos")
            sin_sb = p1c.tile([128, S], F32, tag="sin")
            nc.sync.dma_start(cos_sb[:], cos4.ap())
            nc.sync.dma_start(sin_sb[:], sin4.ap())
            wq_sb_A = p1c.tile([128, DT, 128], BF16, tag="wqA")
            wq_sb_B = p1c.tile([128, DT, 128], BF16, tag="wqB")
            wkv_sb = p1c.tile([128, DT, 128], BF16, tag="wkv")
            nc.gpsimd.dma_start(wq_sb_A[:],
                                wqTA.ap().rearrange("(dt p) e -> p dt e", p=128))
            nc.gpsimd.dma_start(wq_sb_B[:],
                                wqTB.ap().rearrange("(dt p) e -> p dt e", p=128))
            nc.gpsimd.dma_start(wkv_sb[:],
                                wkvT.ap().rearrange("(dt p) e -> p dt e", p=128))

            for tb in range(NTB):
                t0 = tb * TB
                xbf = []
                for i in range(4):
                    xt_ = xbfp.tile([128, D], BF16, tag="xbf")
                    nc.gpsimd.dma_start(xt_[:], x[t0 + 128 * i: t0 + 128 * (i + 1), :])
                    xbf.append(xt_)

                psQA = p1ps.tile([128, TB], F32, tag="psQA")
                psQB = p1ps.tile([128, TB], F32, tag="psQB")
                psKV = p1ps.tile([128, TB], F32, tag="psKV")

                xT = [None] * DT

                def do_transpose(dt):
                    psT = p1pst.tile([128, TB], BF16, tag="psT")
                    for i in range(4):
                        nc.tensor.transpose(
                            psT[:, 128 * i: 128 * (i + 1)],
                            xbf[i][:, 128 * dt: 128 * (dt + 1)], ident[:])
                    xt_ = xtp.tile([128, TB], BF16, tag="xT")
                    nc.vector.tensor_copy(xt_[:], psT[:])
                    return xt_

                xT[0] = do_transpose(0)
                for dt in range(DT):
                    if dt + 1 < DT:
                        xT[dt + 1] = do_transpose(dt + 1)
                    st = dict(start=(dt == 0), stop=(dt == DT - 1))
                    nc.tensor.matmul(psQA[:], wq_sb_A[:, dt, :], xT[dt][:], **st)
                    nc.tensor.matmul(psQB[:], wq_sb_B[:, dt, :], xT[dt][:], **st)
                    nc.tensor.matmul(psKV[:], wkv_sb[:, dt, :], xT[dt][:], **st)

                s0 = t0 % S
                cs = cos_sb[:, s0:s0 + TB]
                sn = sin_sb[:, s0:s0 + TB]
                t1 = p1sb.tile([128, TB], F32, tag="t1")
                t2 = p1sb.tile([128, TB], F32, tag="t2")
                t3 = p1sb.tile([128, TB], F32, tag="t3")
                t4 = p1sb.tile([128, TB], F32, tag="t4")
                nc.vector.tensor_mul(t1[:], psQA[:], cs)
                nc.vector.tensor_mul(t2[:], psQB[:], sn)
                nc.vector.tensor_mul(t3[:], psQA[:], sn)
                nc.vector.tensor_mul(t4[:], psQB[:], cs)
                Aout = p1sb.tile([128, TB], BF16, tag="Aout")
                Bout = p1sb.tile([128, TB], BF16, tag="Bout")
                nc.vector.tensor_sub(Aout[:], t1[:], t2[:])
                nc.vector.tensor_add(Bout[:], t3[:], t4[:])
                for h in range(HPC):
                    nc.vector.tensor_copy(QT[h][0:32, t0:t0 + TB],
                                          Aout[32 * h:32 * (h + 1), :])
                    nc.vector.tensor_copy(QT[h][32:64, t0:t0 + TB],
                                          Bout[32 * h:32 * (h + 1), :])
                k1 = p1sb.tile([32, TB], F32, tag="k1")
                k2 = p1sb.tile([32, TB], F32, tag="k2")
                k3 = p1sb.tile([32, TB], F32, tag="k3")
                k4 = p1sb.tile([32, TB], F32, tag="k4")
                nc.vector.tensor_mul(k1[:], psKV[0:32, :], cs[0:32, :])
                nc.vector.tensor_mul(k2[:], psKV[32:64, :], sn[0:32, :])
                nc.vector.tensor_mul(k3[:], psKV[0:32, :], sn[0:32, :])
                nc.vector.tensor_mul(k4[:], psKV[32:64, :], cs[0:32, :])
                nc.vector.tensor_sub(KT[0:32, t0:t0 + TB], k1[:], k2[:])
                nc.vector.tensor_add(KT[32:64, t0:t0 + TB], k3[:], k4[:])

                vst = p1sb.tile([64, TB], BF16, tag="vst")
                nc.scalar.copy(vst[:], psKV[64:128, :])
                psV = p1pst.tile([128, 4 * 64], BF16, tag="psT")
                for i in range(4):
                    nc.tensor.transpose(psV[:, 64 * i:64 * (i + 1)],
                                        vst[:, 128 * i:128 * (i + 1)],
                                        ident[0:64, 0:64])
                for i in range(4):
                    nc.vector.tensor_copy(V_aug[tb * 4 + i][:, 0:64],
                                          psV[:, 64 * i:64 * (i + 1)])

        # prefetch woT while attention runs
        wo_pool = es.enter_context(tc.tile_pool(name="wo", bufs=DT))
        wo_sb = []
        for dt in range(DT):
            w = wo_pool.tile([128, D], BF16, tag="wo")
            nc.gpsimd.dma_start(w[:], woT[128 * dt:128 * (dt + 1), :])
            wo_sb.append(w)

        # ---------- phase 2: attention ----------
        with tc.tile_pool(name="att", bufs=2) as att, \
             tc.tile_pool(name="expp", bufs=2) as expp, \
             tc.tile_pool(name="psS", bufs=2, space="PSUM") as psSp, \
             tc.tile_pool(name="psO", bufs=2, space="PSUM") as psOp, \
             tc.tile_pool(name="psAT", bufs=2, space="PSUM") as psATp:
            for b in range(B):
                for h in range(HPC):
                    qrows = QT[h]
                    expS = []
                    for kt in range(KTILES):
                        width = S - 128 * kt
                        e = expp.tile([128, width], BF16, tag=f"expS{kt}",
                                      name=f"expS{kt}")
                        expS.append(e)
                        klhs = KT[:, b * S + 128 * kt: b * S + 128 * (kt + 1)]
                        for s0 in range(128 * kt, S, QSPAN):
                            w = min(QSPAN, S - s0)
                            ps = psSp.tile([128, QSPAN], F32, tag="psS")
                            for n0 in range(0, w, 512):
                                nw = min(512, w - n0)
                                nc.tensor.matmul(
                                    ps[:, n0:n0 + nw], klhs,
                                    qrows[:, b * S + s0 + n0: b * S + s0 + n0 + nw],
                                    start=True, stop=True)
                            nc.scalar.activation(
                                e[:, s0 - 128 * kt: s0 - 128 * kt + w],
                                ps[:, 0:w], AF.Exp, scale=0.125)
                        nc.gpsimd.affine_select(
                            out=e[:, 0:128], in_=e[:, 0:128],
                            compare_op=mybir.AluOpType.is_ge, fill=0.0,
                            base=0, pattern=[[1, 128]], channel_multiplier=-1)

                    attnT = att.tile([64, S], BF16, tag="attnT")
                    for qt in range(KTILES):
                        psO = psOp.tile([128, 65], F32, tag="psO")
                        for i in range(qt + 1):
                            nc.tensor.matmul(
                                psO[:],
                                expS[i][:, 128 * (qt - i): 128 * (qt - i) + 128],
                                V_aug[b * KTILES + i][:],
                                start=(i == 0), stop=(i == qt))
                        rc = att.tile([128, 1], F32, tag="rc")
                        nc.vector.reciprocal(rc[:], psO[:, 64:65])
                        attn_n = att.tile([128, 64], BF16, tag="attn_n")
                        nc.vector.tensor_scalar(attn_n[:], psO[:, 0:64], rc[:],
                                                None, mybir.AluOpType.mult)
                        psAT = psATp.tile([64, 128], BF16, tag="psAT")
                        nc.tensor.transpose(psAT[:], attn_n[:], ident[:])
                        nc.vector.tensor_copy(attnT[:, 128 * qt:128 * (qt + 1)],
                                              psAT[:])
                    for j in range(NC_CORES):
                        nc.sync.dma_start(
                            a2a_in[b][j, HD * h:HD * (h + 1), :],
                            attnT[:, BSL * j:BSL * (j + 1)])
                nc.gpsimd.collective_compute(
                    "AllToAll", mybir.AluOpType.bypass, replica_groups=rg,
                    ins=[a2a_in[b][:]], outs=[a2a_out[b][:]])

        # ---------- phase 3: wo ----------
        with tc.tile_pool(name="p3sb", bufs=4) as p3sb, \
             tc.tile_pool(name="rcv", bufs=2 * DT) as rcvp, \
             tc.tile_pool(name="psW", bufs=8, space="PSUM") as psWp:
            for b in range(B):
                rcv = []
                for dt in range(DT):
                    r = rcvp.tile([128, BSL], BF16, tag="rcv")
                    nc.sync.dma_start(
                        r[:],
                        a2a_out[b][dt // 2, (dt % 2) * 128:(dt % 2) * 128 + 128, :])
                    rcv.append(r)
                for tt in range(BSL // 128):
                    psW = [psWp.tile([128, 512], F32, tag="psW", name=f"psW{i}")
                           for i in range(4)]
                    for dt in range(DT):
                        for eb in range(4):
                            nc.tensor.matmul(
                                psW[eb][:],
                                rcv[dt][:, 128 * tt:128 * (tt + 1)],
                                wo_sb[dt][:, 512 * eb:512 * (eb + 1)],
                                start=(dt == 0), stop=(dt == DT - 1))
                    for eb in range(4):
                        osb = p3sb.tile([128, 512], F32, tag="osb")
                        nc.scalar.copy(osb[:], psW[eb][:])
                        nc.sync.dma_start(
                            out[b * BSL + 128 * tt: b * BSL + 128 * (tt + 1),
                                512 * eb:512 * (eb + 1)],
                            osb[:])

    nc.compile()
    return nc


def _perm_eo(n):
    return list(range(0, n, 2)) + list(range(1, n, 2))


def host_inputs(x, freqs_cos, freqs_sin, wq, wk, wv, wo):
    x2d = np.ascontiguousarray(np.asarray(x).reshape(T, D), dtype=np.float32)
    fcT = np.asarray(freqs_cos).T.astype(np.float32)
    fsT = np.asarray(freqs_sin).T.astype(np.float32)
    cos4 = np.ascontiguousarray(np.tile(fcT, (4, 1)))
    sin4 = np.ascontiguousarray(np.tile(fsT, (4, 1)))
    woT = np.ascontiguousarray(np.asarray(wo).T, dtype=np.float32)
    wq = np.asarray(wq)
    wk = np.asarray(wk)
    wv = np.asarray(wv)

    permA = [h * HD + 2 * j for h in range(HPC) for j in range(HD // 2)]
    permB = [h * HD + 2 * j + 1 for h in range(HPC) for j in range(HD // 2)]
    permK = _perm_eo(HD)

    in_maps = []
    for c in range(NC_CORES):
        wq_c = wq[EQ * c: EQ * (c + 1)]
        wqTA = np.ascontiguousarray(wq_c[permA].T, dtype=np.float32)
        wqTB = np.ascontiguousarray(wq_c[permB].T, dtype=np.float32)
        wk_c = wk[HD * c: HD * (c + 1)]
        wv_c = wv[HD * c: HD * (c + 1)]
        wkvT = np.ascontiguousarray(
            np.concatenate([wk_c[permK], wv_c], axis=0).T, dtype=np.float32)
        in_maps.append({
            "x": x2d, "cos4": cos4, "sin4": sin4,
            "wqTA": wqTA, "wqTB": wqTB, "wkvT": wkvT, "woT": woT,
        })
    return in_maps


def host_gather(results):
    full = np.zeros((B, S, D), np.float32)
    for c in range(NC_CORES):
        o = results[c]["out"]
        for b in range(B):
            full[b, BSL * c: BSL * (c + 1), :] = o[b * BSL:(b + 1) * BSL]
    return full


_NC_CACHE = None


def _get_nc():
    global _NC_CACHE
    if _NC_CACHE is None:
        _NC_CACHE = build()
    return _NC_CACHE


def kernel(x, freqs_cos, freqs_sin, wq, wk, wv, wo):
    nc = _get_nc()
    in_maps = host_inputs(x, freqs_cos, freqs_sin, wq, wk, wv, wo)
    res = run_bass_kernel_spmd(nc, in_maps, core_ids=list(range(NC_CORES)))
    return host_gather(res.results)
